# revision 32
# baseline (speedup 1.0000x reference)
"""Multi-head attention (B=2, S=2048, H=2048, 16 heads, d=128) on 8 TRN2
NeuronCores.

Sharding: 2-way batch x 4-way head-group tensor parallel. Core j handles
batch j//4 and heads 4*(j%4)..4*(j%4)+3 (a 512-wide slice of the qkv
projection output dim / o_proj input dim). Each core returns a partial
o_proj output [S, H] in fp16; the host sums the 4 partials per batch and
adds bo.

v2 layout: x stays resident in SBUF (8MB, no per-head re-streaming), all
HBM loads are >=64KB pair-chunk DMAs (the Sync HWDGE ring issues one
dma_start per ~0.6us, so many small chunks cap effective bandwidth),
weights use per-head tiles, and the o_proj output is stored as fp16
[128,2048] row tiles (one DMA per seq tile). P1 interleaves the v/q/k
projection chains chunk-by-chunk across 6 PSUM banks so the PE tracks
DMA arrival instead of stalling on full-tile loads (keeps the HAM clock
warm). qT/kT live in 2-deep per-head rings; exp lives in an 8-slot ring.

On-device compute per core (fp16 matmul operands, fp32 PSUM accumulation):
  P1: v = xT.T @ wvT (+bias) and head 0's qT/kT projections, interleaved
      per chunk across ps_a/ps_sum/ps_ctx (v) + ps_sc (q/k).
  P2: for h in 0..2: attention(h) stages with head h+1's qT/kT projection
      matmuls as per-stage PE filler (x read from resident SBUF tiles).
  P3: attention(h=3) with o_proj matmuls of completed q-blocks as filler.
  P4: leftover o_proj + output staging.

Attention per (head, q-block), scoresT layout [k, q]:
  scoresT = kT_h.T @ qT_h -> exp on ScalarE -> expT (fp16, 8-slot ring)
  VectorE pair-adds exp chunks; sums += ones128.T @ pair  (half-cost rowsum)
  ctxT += v_chunk.T @ expT ; ctxT_norm = ctxT * approx_recip(sums)
"""
import sys

if "/opt/trn_rl_repo" not in sys.path:
    sys.path.insert(0, "/opt/trn_rl_repo")

import numpy as np

HIDDEN = 2048
HEADS = 16
HEAD_DIM = 128
BATCH = 2
SEQ = 2048

N_CORES = 8
GROUPS = 4               # head groups (cores per batch)
GDIM = HIDDEN // GROUPS  # 512 dims per core
GHEADS = GDIM // HEAD_DIM  # 4 heads per core
KC = HIDDEN // 128       # 16 contraction chunks
KP = KC // 2             # 8 pair-chunks (DMA granularity)
SB = 4                   # seq blocks of 512
QB = SEQ // 512          # 4 q-blocks in attention
MT = SEQ // 128          # 16 seq tiles of 128
ERS = 8                  # eblk ring slots (of 512 cols each)

_CACHE = {}


def _build():
    import concourse.bacc as bacc
    import concourse.bass as bass
    import concourse.mybir as mybir
    import concourse.tile as tile

    fp16 = mybir.dt.float16
    fp32 = mybir.dt.float32
    AF = mybir.ActivationFunctionType

    nc = bacc.Bacc("TRN2", target_bir_lowering=False, debug=False,
                   num_devices=N_CORES)

    xT = nc.dram_tensor("xt", [SB, KP, 128, 1024], fp16, kind="ExternalInput").ap()
    wqh = nc.dram_tensor("wqh", [GHEADS, KP, 128, 256], fp16, kind="ExternalInput").ap()
    wkh = nc.dram_tensor("wkh", [GHEADS, KP, 128, 256], fp16, kind="ExternalInput").ap()
    wvT = nc.dram_tensor("wvt", [KP, 128, 1024], fp16, kind="ExternalInput").ap()
    woT = nc.dram_tensor("wot", [GHEADS, 128, HIDDEN], fp16, kind="ExternalInput").ap()
    bqkd = nc.dram_tensor("bqk", [128, 2 * GHEADS], fp32, kind="ExternalInput").ap()
    out = nc.dram_tensor("out", [MT, 128, HIDDEN], fp16, kind="ExternalOutput").ap()

    with tile.TileContext(nc) as tc:
        with (
            tc.tile_pool(name="xp", bufs=4) as xp,          # 4 x 2MB resident x
            tc.tile_pool(name="wqkp", bufs=4) as wqkp,      # 4 x 0.5MB w slices
            tc.tile_pool(name="wvop", bufs=1) as wvop,      # 2MB: wv then wo
            tc.tile_pool(name="qtp", bufs=2) as qtp,        # per-head qT ring
            tc.tile_pool(name="ktp", bufs=2) as ktp,        # per-head kT ring
            tc.tile_pool(name="res", bufs=1) as res,        # v, ctx (2MB each)
            tc.tile_pool(name="ebp", bufs=1) as ebp,        # 1MB exp ring
            tc.tile_pool(name="epp", bufs=2) as epp,        # 2 x 1MB pair sums
            tc.tile_pool(name="small", bufs=1) as small,
            tc.tile_pool(name="rec", bufs=1) as rec,
            tc.tile_pool(name="outp", bufs=3) as outp,      # 3 x 0.5MB out rows
            tc.tile_pool(name="ps_a", bufs=2, space=bass.MemorySpace.PSUM) as ps_a,
            tc.tile_pool(name="ps_sc", bufs=2, space=bass.MemorySpace.PSUM) as ps_sc,
            tc.tile_pool(name="ps_sum", bufs=2, space=bass.MemorySpace.PSUM) as ps_sum,
            tc.tile_pool(name="ps_ctx", bufs=2, space=bass.MemorySpace.PSUM) as ps_ctx,
        ):
            xv = [xp.tile([128, KC * 512], fp16, tag="x", name=f"xv{s}")
                  for s in range(SB)]
            wq_h = [wqkp.tile([128, KC * 128], fp16, tag="wqk", name=f"wq{h}")
                    for h in range(GHEADS)]
            wk_h = [wqkp.tile([128, KC * 128], fp16, tag="wqk", name=f"wk{h}")
                    for h in range(GHEADS)]
            qT_h = [qtp.tile([128, SEQ], fp16, tag="qT", name=f"qT{h}")
                    for h in range(GHEADS)]
            kT_h = [ktp.tile([128, SEQ], fp16, tag="kT", name=f"kT{h}")
                    for h in range(GHEADS)]
            wv_sb = wvop.tile([128, KC * GDIM], fp16, tag="wvo", name="wv_sb")

            v_sb = res.tile([128, MT * GDIM], fp16, tag="v")
            ctx_sb = res.tile([128, GHEADS * SEQ], fp16, tag="ctx")

            eblk = ebp.tile([128, ERS * 512], fp16, tag="eblk")

            bqk_sb = small.tile([128, 2 * GHEADS], fp32, tag="bqk")
            ones_sb = small.tile([128, 128], fp16, tag="ones")
            nc.gpsimd.memset(ones_sb[:], 1.0)  # only needed by rowsums (~95us)

            # ---------- P0: DMA issue order (Sync ring is FIFO) ----------
            # sb0's pairs first so P1 compute chases arrival. Each dma_start
            # costs ~0.6us of ring-issue time, so sb0's weight slices + bias
            # ride the second HWDGE ring (Scalar engine, idle until ~30us)
            # and only wv/x contend on the Sync ring. Heads 1-3's weight
            # loads must NOT go on the Scalar ring: their tile slots free
            # only mid-kernel and a waiting DMA would block ACT's FIFO
            # (and every exp behind it).
            for c2 in range(KP):
                if c2 == 0:
                    # split the very first pair into singles so the first
                    # chunk's matmuls start one transfer earlier
                    for i in (0, 1):
                        nc.sync.dma_start(wv_sb[:, i * 512:(i + 1) * 512],
                                          wvT[0][:, i * 512:(i + 1) * 512])
                        nc.sync.dma_start(xv[0][:, i * 512:(i + 1) * 512],
                                          xT[0, 0][:, i * 512:(i + 1) * 512])
                else:
                    nc.sync.dma_start(wv_sb[:, c2 * 1024:(c2 + 1) * 1024],
                                      wvT[c2])
                    nc.sync.dma_start(xv[0][:, c2 * 1024:(c2 + 1) * 1024],
                                      xT[0, c2])
                nc.scalar.dma_start(wq_h[0][:, c2 * 256:(c2 + 1) * 256],
                                    wqh[0, c2])
                nc.scalar.dma_start(wk_h[0][:, c2 * 256:(c2 + 1) * 256],
                                    wkh[0, c2])
            nc.scalar.dma_start(bqk_sb[:], bqkd)
            for sb in range(1, SB):
                for c2 in range(KP):
                    nc.sync.dma_start(xv[sb][:, c2 * 1024:(c2 + 1) * 1024],
                                      xT[sb, c2])
            for h in range(1, GHEADS):
                for c2 in range(KP):
                    nc.sync.dma_start(wq_h[h][:, c2 * 256:(c2 + 1) * 256],
                                      wqh[h, c2])
                    nc.sync.dma_start(wk_h[h][:, c2 * 256:(c2 + 1) * 256],
                                      wkh[h, c2])

            # No HAM warmup: engine preamble + table loads keep the PE shut
            # until ~4.7us and the first pairs land ~6.3us; real chunk-chase
            # matmuls keep the PE continuously busy from there, so HAM flips
            # by ~10us with only a few half-rate matmuls.

            # ---------- P1: v + head-0 q/k, chunk-interleaved ----------
            for sb in range(SB):
                psv = [ps_a.tile([128, 512], fp32, tag="ps_a", name=f"v{sb}_0"),
                       ps_a.tile([128, 512], fp32, tag="ps_a", name=f"v{sb}_1"),
                       ps_sum.tile([128, 512], fp32, tag="ps_sum", name=f"v{sb}_2"),
                       ps_ctx.tile([128, 512], fp32, tag="ps_ctx", name=f"v{sb}_3")]
                psq = ps_sc.tile([128, 512], fp32, tag="ps_sc", name=f"q{sb}")
                psk = ps_sc.tile([128, 512], fp32, tag="ps_sc", name=f"k{sb}")
                for c in range(KC):
                    for t in range(4):
                        nc.tensor.matmul(
                            psv[t][:],
                            xv[sb][:, c * 512 + t * 128: c * 512 + (t + 1) * 128],
                            wv_sb[:, c * GDIM:(c + 1) * GDIM],
                            start=(c == 0), stop=(c == KC - 1))
                    nc.tensor.matmul(
                        psq[:], wq_h[0][:, c * 128:(c + 1) * 128],
                        xv[sb][:, c * 512:(c + 1) * 512],
                        start=(c == 0), stop=(c == KC - 1))
                    nc.tensor.matmul(
                        psk[:], wk_h[0][:, c * 128:(c + 1) * 128],
                        xv[sb][:, c * 512:(c + 1) * 512],
                        start=(c == 0), stop=(c == KC - 1))
                s0 = sb * 512
                nc.scalar.activation(qT_h[0][:, s0:s0 + 512], psq[:],
                                     AF.Identity, bias=bqk_sb[:, 0:1])
                nc.scalar.activation(kT_h[0][:, s0:s0 + 512], psk[:],
                                     AF.Identity, bias=bqk_sb[:, GHEADS:GHEADS + 1])
                for t in range(4):
                    st = sb * 4 + t
                    nc.vector.tensor_copy(v_sb[:, st * GDIM:(st + 1) * GDIM],
                                          psv[t][:])

            # ---------- P2/P3: attention windows with PE filler ----------
            state = {}
            pend = []

            def drain(bi, kp):
                h, qb, ep, ctxp = state[bi]
                for kc in (2 * kp, 2 * kp + 1):
                    slot = kc % ERS
                    nc.tensor.matmul(ctxp[:],
                                     v_sb[:, kc * GDIM + h * 128:
                                          kc * GDIM + (h + 1) * 128],
                                     eblk[:, slot * 512:(slot + 1) * 512],
                                     start=(kc == 0), stop=(kc == KC - 1))
                if kp == KP - 1:
                    # sums allocated lazily so ps_sum slots stay free for
                    # the o_proj filler rotation most of the block
                    sums = ps_sum.tile([128, 512], fp32, tag="ps_sum",
                                       name=f"sums{bi}")
                    nc.tensor.matmul(sums[:], ones_sb[:],
                                     ep[:, 0:512], start=True, stop=True)
                    finish(bi, sums)

            def finish(bi, sums):
                h, qb, ep, ctxp = state.pop(bi)
                q0 = qb * 512
                recip = rec.tile([128, 512], fp32, tag="recip")
                nc.vector.reciprocal_approx_fast(recip[:], sums[:])
                nc.vector.tensor_mul(ctx_sb[:, h * SEQ + q0: h * SEQ + q0 + 512],
                                     ctxp[:], recip[:])

            # filler generators -------------------------------------------
            def proj_filler(h, k_first=False):
                """Yield 128 single-MM closures projecting head h's qT/kT
                from the resident x tiles. With k_first, all kT chains come
                before qT chains so the tail of the generator (late qT
                blocks) can spill past the head h-1 window into head h's
                first attention block as its PE filler."""
                if k_first:
                    parts = [(wk_h[h], GHEADS, kT_h[h], "k", sb)
                             for sb in range(SB)]
                    parts += [(wq_h[h], 0, qT_h[h], "q", sb)
                              for sb in range(SB)]
                else:
                    parts = [p for sb in range(SB)
                             for p in ((wq_h[h], 0, qT_h[h], "q", sb),
                                       (wk_h[h], GHEADS, kT_h[h], "k", sb))]
                for w_h, boff, dst, nm, sb in parts:
                    s0 = sb * 512
                    ps = ps_a.tile([128, 512], fp32, tag="ps_a",
                                   name=f"p{nm}{h}_{sb}")
                    for c in range(KC):
                        def mm(c=c, ps=ps, w_h=w_h, boff=boff, dst=dst,
                               h=h, sb=sb, s0=s0):
                            nc.tensor.matmul(
                                ps[:],
                                w_h[:, c * 128:(c + 1) * 128],
                                xv[sb][:, c * 512:(c + 1) * 512],
                                start=(c == 0), stop=(c == KC - 1))
                            if c == KC - 1:
                                nc.scalar.activation(
                                    dst[:, s0:s0 + 512],
                                    ps[:], AF.Identity,
                                    bias=bqk_sb[:, boff + h:boff + h + 1])
                        yield mm

            def oproj_filler(qb, pools=None, split_stores=False):
                """Yield 64 single-MM closures for o_proj q-tiles of block qb
                (all heads' ctx for qb must be finished). Output rows are
                staged as fp16 [128, 2048] and stored with one DMA per mq
                (or one per 512-col quarter when split_stores, so the final
                store's completion receipt isn't behind a 512KB transfer)."""
                pools = pools or [(ps_a, "ps_a")]
                o_t = {}
                for i, (mq, oc) in enumerate(
                        (mq, oc) for mq in range(qb * 4, qb * 4 + 4)
                        for oc in range(4)):
                        pool, ptag = pools[i % len(pools)]
                        ps = pool.tile([128, 512], fp32, tag=ptag,
                                       name=f"po{mq}_{oc}")
                        for hh in range(GHEADS):
                            def mm(ps=ps, hh=hh, mq=mq, oc=oc):
                                nc.tensor.matmul(
                                    ps[:],
                                    ctx_sb[:, hh * SEQ + mq * 128:
                                           hh * SEQ + (mq + 1) * 128],
                                    wo_sb[:, hh * HIDDEN + oc * 512:
                                          hh * HIDDEN + (oc + 1) * 512],
                                    start=(hh == 0), stop=(hh == GHEADS - 1))
                                if hh == GHEADS - 1:
                                    if oc == 0:
                                        o_t[mq] = outp.tile(
                                            [128, HIDDEN], fp16, tag="out",
                                            name=f"o_{mq}")
                                    nc.vector.tensor_copy(
                                        o_t[mq][:, oc * 512:(oc + 1) * 512],
                                        ps[:])
                                    if split_stores:
                                        nc.sync.dma_start(
                                            out[mq][:, oc * 512:(oc + 1) * 512],
                                            o_t[mq][:, oc * 512:(oc + 1) * 512])
                                        if oc == 3:
                                            o_t.pop(mq)
                                    elif oc == 3:
                                        nc.sync.dma_start(out[mq],
                                                          o_t.pop(mq)[:])
                            yield mm

            bi = 0
            fillers = []

            def take(n):
                while n > 0 and fillers:
                    mm = next(fillers[0], None)
                    if mm is None:
                        fillers.pop(0)
                        continue
                    mm()
                    n -= 1

            for h in range(GHEADS):
                if h < GHEADS - 2:
                    fillers.append(proj_filler(h + 1))
                    budget = lambda st: 4          # 128 over 32 stages
                elif h == GHEADS - 2:
                    # k-first so the last 24 qT matmuls of head 3 spill past
                    # this window into head 3's first (otherwise filler-less)
                    # attention block
                    fillers.append(proj_filler(h + 1, k_first=True))
                    budget = lambda st: 4 if st < 8 else 3   # 104 of 128
                else:
                    wo_sb = wvop.tile([128, GHEADS * HIDDEN], fp16,
                                      tag="wvo", name="wo_sb")
                    for c in range(GHEADS):
                        nc.sync.dma_start(
                            wo_sb[:, c * HIDDEN:(c + 1) * HIDDEN], woT[c])
                    # qb0 drains the 24 held-back qT matmuls at 3 per stage
                    budget = lambda st: 3
                for qb in range(QB):
                    if h == GHEADS - 1 and qb >= 1:
                        fillers.append(
                            oproj_filler(qb - 1, pools=[(ps_a, "ps_a"),
                                                        (ps_sum, "ps_sum"),
                                                        (ps_ctx, "ps_ctx")]))
                        budget = lambda st: 8
                    q0 = qb * 512
                    ep = epp.tile([128, KP * 512], fp16, tag="ep")
                    ctxp = ps_ctx.tile([128, 512], fp32, tag="ps_ctx")
                    state[bi] = (h, qb, ep, ctxp)
                    for kp in range(KP):
                        for i in (0, 1):
                            kc = 2 * kp + i
                            slot = kc % ERS
                            sc = ps_sc.tile([128, 512], fp32, tag="ps_sc")
                            nc.tensor.matmul(
                                sc[:],
                                kT_h[h][:, kc * 128:(kc + 1) * 128],
                                qT_h[h][:, q0:q0 + 512],
                                start=True, stop=True)
                            nc.scalar.activation(
                                eblk[:, slot * 512:(slot + 1) * 512], sc[:],
                                AF.Exp)
                        sl0 = (2 * kp) % ERS
                        nc.vector.tensor_add(
                            ep[:, kp * 512:(kp + 1) * 512],
                            eblk[:, sl0 * 512:(sl0 + 1) * 512],
                            eblk[:, (sl0 + 1) * 512:(sl0 + 2) * 512])
                        if kp % 2 == 1:
                            nc.vector.tensor_add(
                                ep[:, (kp - 1) * 512: kp * 512],
                                ep[:, (kp - 1) * 512: kp * 512],
                                ep[:, kp * 512:(kp + 1) * 512])
                        if kp % 4 == 3:
                            nc.vector.tensor_add(
                                ep[:, (kp - 3) * 512:(kp - 2) * 512],
                                ep[:, (kp - 3) * 512:(kp - 2) * 512],
                                ep[:, (kp - 1) * 512: kp * 512])
                        if kp == KP - 1:
                            nc.vector.tensor_add(
                                ep[:, 0:512], ep[:, 0:512],
                                ep[:, 4 * 512:5 * 512])
                        for b_kp in pend:
                            drain(*b_kp)
                        pend = [(bi, kp)]
                        take(budget(qb * KP + kp))
                    bi += 1
                if h < GHEADS - 2:
                    take(10 ** 9)  # flush: this head's proj must be complete
            take(10 ** 9)  # defensive: never drop filler work
            for b_kp in pend:
                drain(*b_kp)

            # ---------- P4: leftover o_proj (last q-block) ----------
            # attention PSUM pools are idle now; rotate across them so the
            # PE never waits on a copy to release a bank
            for mm in oproj_filler(QB - 1, pools=[(ps_a, "ps_a"),
                                                  (ps_sum, "ps_sum"),
                                                  (ps_ctx, "ps_ctx")],
                                   split_stores=True):
                mm()

    nc.compile()
    return nc


def kernel(x, wq, bq, wk, bk, wv, bv, wo, bo):
    from concourse import bass_utils

    if "nc" not in _CACHE:
        _CACHE["nc"] = _build()
    nc = _CACHE["nc"]

    x = np.asarray(x, np.float32)
    scale = np.float32(1.0 / np.sqrt(HEAD_DIM))

    def pair3(a):  # [H, N] (contraction-major) -> [KP, 128, 2N] pair layout
        n = a.shape[1]
        return np.ascontiguousarray(
            a.reshape(KP, 2, 128, n).transpose(0, 2, 1, 3).reshape(KP, 128, 2 * n)
        ).astype(np.float16)

    xT = []
    for b in range(BATCH):
        xb = np.ascontiguousarray(x[b].T)  # [H, S]
        xT.append(np.ascontiguousarray(
            xb.reshape(KP, 2, 128, SB, 512).transpose(3, 0, 2, 1, 4)
            .reshape(SB, KP, 128, 1024)).astype(np.float16))

    in_maps = []
    for j in range(N_CORES):
        b, g = divmod(j, GROUPS)
        ds = slice(g * GDIM, (g + 1) * GDIM)
        wq_g = np.asarray(wq)[ds] * scale  # [512, H]
        wk_g = np.asarray(wk)[ds]
        wv_g = np.asarray(wv)[ds]
        bqk = np.concatenate([
            (np.asarray(bq)[ds] * scale).reshape(GHEADS, 128).T,
            np.asarray(bk)[ds].reshape(GHEADS, 128).T], axis=1)
        in_maps.append({
            "xt": xT[b],
            "wqh": np.stack([pair3(np.ascontiguousarray(
                wq_g[h * 128:(h + 1) * 128].T)) for h in range(GHEADS)]),
            "wkh": np.stack([pair3(np.ascontiguousarray(
                wk_g[h * 128:(h + 1) * 128].T)) for h in range(GHEADS)]),
            "wvt": pair3(np.ascontiguousarray(wv_g.T)),
            "wot": np.ascontiguousarray(np.asarray(wo)[:, ds].T).astype(
                np.float16).reshape(GHEADS, 128, HIDDEN),
            "bqk": np.ascontiguousarray(bqk).astype(np.float32),
        })

    res = bass_utils.run_bass_kernel_spmd(
        nc, in_maps, core_ids=list(range(N_CORES)),
        **_CACHE.get("run_kwargs", {}))
    _CACHE["last_res"] = res

    out = np.zeros((BATCH, MT, 128, HIDDEN), np.float32)
    for j in range(N_CORES):
        b = j // GROUPS
        out[b] += res.results[j]["out"].astype(np.float32)
    out = out.reshape(BATCH, SEQ, HIDDEN)
    # bv passes through the softmax average linearly (weights sum to 1),
    # so its o_proj image folds into the output bias on the host.
    bo_eff = np.asarray(bo, np.float32) + (
        np.asarray(bv, np.float32) @ np.asarray(wo, np.float32).T)
    out = out + bo_eff
    return out
